# revision 7
# baseline (speedup 1.0000x reference)
"""Causal self-attention (GPT-style block) on 8 Trainium2 NeuronCores.

Problem: x[4, 2048, 768], w_attn[2304, 768], b_attn[2304], w_proj[768, 768],
b_proj[768]; 12 heads of size 64; causal softmax attention; output [4, 2048, 768].

Sharding: batch x heads. core = 2*b + g handles batch b (of 4) and the 6 heads
g*6..g*6+5 (tensor parallel over heads). Each core:
  1. QKV projection for its head slice, producing Q^T/K^T in [r, t] layout and
     V in [t, r] layout (plus a fused ones column for softmax denominators).
  2. Flash-style causal attention per head: S^T tiles [128 kv, 512 q] via PE,
     exp on ACT (scale=1/8), triangular mask on the diagonal 128x128 block via
     DVE, O^T accumulation on PE with the V-ones column yielding the softmax
     denominator for free, then per-column normalization via a rank-1
     broadcast matmul + DVE multiply.
  3. c_proj with its 384 local channels -> partial y[2048, 768].
  4. Paired ReduceScatter (cores 2b, 2b+1) sums the partials; each core ends
     with half the rows of y[b]. Host reassembles and adds b_proj.

All matmuls run as float32r (TF32-like, 1 cycle/row at N>=256) with fp32 PSUM
accumulation.
"""
import os

import numpy as np

os.environ.setdefault("JAX_COMPILATION_CACHE_DIR", "/tmp/jaxcache")
os.environ.setdefault("JAX_PERSISTENT_CACHE_MIN_COMPILE_TIME_SECS", "0")
os.environ.setdefault("JAX_PERSISTENT_CACHE_MIN_ENTRY_SIZE_BYTES", "0")

import concourse.bass as bass
import concourse.bacc as bacc
import concourse.tile as tile
from concourse import mybir
from concourse.bass_utils import run_bass_kernel_spmd

B, T, C, H = 4, 2048, 768, 12
HS = 64          # head size
HL = 6           # heads per core
CL = HL * HS     # 384 local channels per core
NQ = 512         # q block width
NCH = T // NQ    # 4 chunks
NKB = T // 128   # 16 kv blocks
NCORES = 8
F32 = mybir.dt.float32
F32R = mybir.dt.float32r
EXP = mybir.ActivationFunctionType.Exp
GROUPS = [[0, 1], [2, 3], [4, 5], [6, 7]]


def build_bass():
    nc = bacc.Bacc(num_devices=NCORES)
    xT = nc.declare_dram_parameter("xT", [C, T], F32, isOutput=False)
    wqkT = nc.declare_dram_parameter("wqkT", [C, 2 * CL], F32, isOutput=False)
    wvT = nc.declare_dram_parameter("wvT", [C, CL], F32, isOutput=False)
    wpT = nc.declare_dram_parameter("wpT", [CL, C], F32, isOutput=False)
    bqk = nc.declare_dram_parameter("bqk", [1, 2 * CL], F32, isOutput=False)
    bv = nc.declare_dram_parameter("bv", [1, CL], F32, isOutput=False)
    tri = nc.declare_dram_parameter("tri", [128, 128], F32, isOutput=False)
    ones = nc.declare_dram_parameter("ones", [128, NQ], F32, isOutput=False)
    out_half = nc.declare_dram_parameter("out_half", [T // 2, C], F32, isOutput=True)

    y_part = nc.dram_tensor("y_part", [T, C], F32)
    rs_out = nc.dram_tensor("rs_out", [T // 2, C], F32)

    with tile.TileContext(nc) as tc:
        with (
            tc.tile_pool(name="const", bufs=1) as constp,
            tc.tile_pool(name="wpool", bufs=1) as wpool,
            tc.tile_pool(name="qkv", bufs=1) as qkvp,
            tc.tile_pool(name="xch", bufs=2) as xchp,
            tc.tile_pool(name="ptp", bufs=4) as ptp,
            tc.tile_pool(name="otsb", bufs=2) as otsbp,
            tc.tile_pool(name="small", bufs=3) as smallp,
            tc.tile_pool(name="yev", bufs=2) as yevp,
            tc.tile_pool(name="ps_big", bufs=4, space="PSUM") as psb,
            tc.tile_pool(name="ps_ot", bufs=2, space="PSUM") as psot,
            tc.tile_pool(name="ps_bc", bufs=1, space="PSUM") as psbc,
        ):
            # ---- constants + weights ----
            tri_sb = constp.tile([128, 128], F32)
            nc.sync.dma_start(out=tri_sb, in_=tri[:, :])
            ones_sb = constp.tile([1, NQ], F32R)
            nc.sync.dma_start(out=ones_sb, in_=ones[0:1, :].bitcast(F32R))
            bqk_sb = constp.tile([1, 2 * CL], F32R)
            nc.sync.dma_start(out=bqk_sb, in_=bqk[:, :].bitcast(F32R))
            bv_sb = constp.tile([1, CL], F32R)
            nc.sync.dma_start(out=bv_sb, in_=bv[:, :].bitcast(F32R))

            wqk_sb = []
            for cb in range(6):
                wt = wpool.tile([128, 2 * CL], F32R, tag=f"wqk{cb}")
                nc.sync.dma_start(
                    out=wt, in_=wqkT[cb * 128:(cb + 1) * 128, :].bitcast(F32R))
                wqk_sb.append(wt)
            wv_sb = []
            for cb in range(6):
                wt = wpool.tile([128, CL], F32R, tag=f"wv{cb}")
                nc.sync.dma_start(
                    out=wt, in_=wvT[cb * 128:(cb + 1) * 128, :].bitcast(F32R))
                wv_sb.append(wt)
            wp_sb = []
            for cb in range(3):
                wt = wpool.tile([128, C], F32R, tag=f"wp{cb}")
                nc.sync.dma_start(
                    out=wt, in_=wpT[cb * 128:(cb + 1) * 128, :].bitcast(F32R))
                wp_sb.append(wt)

            # persistent activations
            QT = [qkvp.tile([128, T], F32R, tag=f"qt{i}", name=f"qt{i}") for i in range(3)]
            KT = [qkvp.tile([128, T], F32R, tag=f"kt{i}", name=f"kt{i}") for i in range(3)]
            V = qkvp.tile([128, NKB, HL, HS + 1], F32R, tag="v")
            nc.sync.dma_start(
                out=V[:, :, :, HS],
                in_=ones[:, 0:NKB * HL].bitcast(F32R).rearrange(
                    "p (a b) -> p a b", b=HL))

            xTr = xT[:, :].bitcast(F32R).rearrange("(cb p) t -> p cb t", p=128)

            # ---- Phase A: QKV projection per t-chunk ----
            for tcn in range(NCH):
                xc = xchp.tile([128, 6, NQ], F32R, tag="xc")
                nc.sync.dma_start(out=xc, in_=xTr[:, :, tcn * NQ:(tcn + 1) * NQ])
                # Q^T / K^T: [r, t] layout, 6 row-blocks (3 Q + 3 K)
                for rb in range(6):
                    ps = psb.tile([128, NQ], F32, tag="big")
                    for cb in range(6):
                        nc.tensor.matmul(
                            ps, lhsT=wqk_sb[cb][:, rb * 128:(rb + 1) * 128],
                            rhs=xc[:, cb, :], start=(cb == 0), stop=False)
                    nc.tensor.matmul(
                        ps, lhsT=bqk_sb[:, rb * 128:(rb + 1) * 128],
                        rhs=ones_sb, start=False, stop=True)
                    dst = QT[rb] if rb < 3 else KT[rb - 3]
                    nc.vector.tensor_copy(dst[:, tcn * NQ:(tcn + 1) * NQ], ps)
                # V: [t, r] layout, 4 t-subblocks
                for tb in range(4):
                    ti = tcn * 4 + tb
                    psv = psb.tile([128, CL], F32, tag="big")
                    for cb in range(6):
                        nc.tensor.matmul(
                            psv, lhsT=xc[:, cb, tb * 128:(tb + 1) * 128],
                            rhs=wv_sb[cb], start=(cb == 0), stop=False)
                    nc.tensor.matmul(
                        psv, lhsT=ones_sb[:, 0:128], rhs=bv_sb,
                        start=False, stop=True)
                    nc.vector.tensor_copy(
                        V[:, ti, :, 0:HS],
                        psv.rearrange("p (h d) -> p h d", d=HS))

            # ---- Phase B: attention + c_proj + collective per q-block ----
            for J in range(NCH):
                qs = slice(J * NQ, (J + 1) * NQ)
                ots = [otsbp.tile([128, NQ], F32R, tag=f"ots{cb}", name=f"ots{cb}") for cb in range(3)]
                for h in range(HL):
                    kb, po = h // 2, (h % 2) * HS
                    qt = QT[kb][po:po + HS, qs]
                    ot = psot.tile([HS + 1, NQ], F32, tag="ot")
                    for t in range(J * 4):           # full kv tiles
                        sps = psb.tile([128, NQ], F32, tag="big")
                        nc.tensor.matmul(
                            sps, lhsT=KT[kb][po:po + HS, t * 128:(t + 1) * 128],
                            rhs=qt, start=True, stop=True)
                        pt = ptp.tile([128, NQ], F32R, tag="pt")
                        nc.scalar.activation(pt, sps, EXP, scale=0.125)
                        nc.tensor.matmul(
                            ot, lhsT=V[:, t, h, :], rhs=pt,
                            start=(t == 0), stop=False)
                    for d in range(4):               # diagonal kv tiles
                        t = J * 4 + d
                        W = NQ - 128 * d
                        sps = psb.tile([128, NQ], F32, tag="big")
                        nc.tensor.matmul(
                            sps[:, 0:W],
                            lhsT=KT[kb][po:po + HS, t * 128:(t + 1) * 128],
                            rhs=QT[kb][po:po + HS, J * NQ + 128 * d:(J + 1) * NQ],
                            start=True, stop=True)
                        pt = ptp.tile([128, NQ], F32R, tag="pt")
                        nc.scalar.activation(pt[:, 0:W], sps[:, 0:W], EXP, scale=0.125)
                        nc.vector.tensor_mul(pt[:, 0:128], pt[:, 0:128], tri_sb)
                        nc.tensor.matmul(
                            ot[:, 128 * d:NQ], lhsT=V[:, t, h, :], rhs=pt[:, 0:W],
                            start=(J == 0 and d == 0), stop=(d == 3))
                    # normalize: recip of denominator row, broadcast via rank-1 mm
                    rec = smallp.tile([1, NQ], F32R, tag="rec")
                    with nc.allow_low_precision(reason="fp32r matmul operand"):
                        nc.vector.reciprocal(rec, ot[HS:HS + 1, :])
                    bc = psbc.tile([HS, NQ], F32, tag="bc")
                    nc.tensor.matmul(bc, lhsT=ones_sb[:, 0:HS], rhs=rec,
                                     start=True, stop=True)
                    bcs = smallp.tile([HS, NQ], F32, tag="bcs")
                    nc.scalar.copy(bcs, bc)
                    nc.vector.tensor_mul(ots[kb][po:po + HS, :], ot[0:HS, :], bcs)
                # c_proj for this q-block
                for i in range(4):
                    ti = J * 4 + i
                    yt = yevp.tile([128, C], F32, tag="yt")
                    for half in range(2):
                        yps = psb.tile([128, CL], F32, tag="big")
                        for cb in range(3):
                            nc.tensor.matmul(
                                yps, lhsT=ots[cb][:, i * 128:(i + 1) * 128],
                                rhs=wp_sb[cb][:, half * CL:(half + 1) * CL],
                                start=(cb == 0), stop=(cb == 2))
                        nc.vector.tensor_copy(yt[:, half * CL:(half + 1) * CL], yps)
                    nc.sync.dma_start(
                        out=y_part[ti * 128:(ti + 1) * 128, :], in_=yt)
                nc.gpsimd.collective_compute(
                    "ReduceScatter", mybir.AluOpType.add, replica_groups=GROUPS,
                    ins=[y_part[J * NQ:(J + 1) * NQ, :]],
                    outs=[rs_out[J * 256:(J + 1) * 256, :]])
                nc.sync.dma_start(
                    out=out_half[J * 256:(J + 1) * 256, :],
                    in_=rs_out[J * 256:(J + 1) * 256, :])
    nc.finalize()
    return nc


def make_in_maps(x, w_attn, b_attn, w_proj):
    x = np.asarray(x, dtype=np.float32)
    w_attn = np.asarray(w_attn, dtype=np.float32)
    b_attn = np.asarray(b_attn, dtype=np.float32)
    w_proj = np.asarray(w_proj, dtype=np.float32)
    # valid iff kv <= q with kv on partitions (rows), q on free dim (cols)
    tri = np.triu(np.ones((128, 128), dtype=np.float32))
    in_maps = []
    for core in range(NCORES):
        b, g = divmod(core, 2)
        sl = slice(g * CL, (g + 1) * CL)
        wq, wk, wv = (w_attn[i * C:(i + 1) * C][sl] for i in range(3))
        bq, bk, bv_ = (b_attn[i * C:(i + 1) * C][sl] for i in range(3))
        in_maps.append({
            "xT": np.ascontiguousarray(x[b].T),
            "wqkT": np.ascontiguousarray(np.concatenate([wq, wk], 0).T),
            "wvT": np.ascontiguousarray(wv.T),
            "wpT": np.ascontiguousarray(w_proj[:, sl].T),
            "bqk": np.concatenate([bq, bk])[None, :].copy(),
            "bv": bv_[None, :].copy(),
            "tri": tri,
            "ones": np.ones((128, NQ), dtype=np.float32),
        })
    return in_maps


def assemble(results, b_proj):
    out = np.empty((B, T, C), dtype=np.float32)
    for core in range(NCORES):
        b, g = divmod(core, 2)
        half = results[core]["out_half"]
        for J in range(NCH):
            out[b, J * NQ + g * 256:J * NQ + (g + 1) * 256, :] = \
                half[J * 256:(J + 1) * 256, :]
    out += np.asarray(b_proj, dtype=np.float32)[None, None, :]
    return out


_CACHE = {}


def _get_nc():
    if "nc" not in _CACHE:
        _CACHE["nc"] = build_bass()
    return _CACHE["nc"]


def kernel(x, w_attn, b_attn, w_proj, b_proj):
    in_maps = make_in_maps(x, w_attn, b_attn, w_proj)
    res = run_bass_kernel_spmd(_get_nc(), in_maps, list(range(NCORES)))
    return assemble(res.results, b_proj)
